# revision 15
# baseline (speedup 1.0000x reference)
"""Trainium2 Bass kernel for nn_DualDescriptorTS.

Math:  Nk[b,i] = sum_{j,g} x[b,j] * P[i,j,g] * cos(2*pi*k[b]/p[i,j,g]),
       p[i,j,g] = i*1024 + j*16 + g + 2,  x = emb[token_indices].

Key identity (k = arange(B), so k_b = b = 32*h + l, h in [0,128),
l in [0,32)): by angle addition, the P-weighted phi slab of each
(i, j) pair is a small-rank product

  D_{i,j}[l, h] = stat^T mov,
  stat[(c,g), l] = {P*cos(l*th_g), -P*sin(l*th_g)},  mov[(c,g), h] =
  {cos(32h*th_g), sin(32h*th_g)},  th_g = 2*pi/p.

mov is P-independent and numerically low rank (the 16 periods of a
slab are nearly equal): mov ~ A @ Q with per-slab rank K (median 2 at
a 1% Frobenius tail).  Q ships as the matmul moving operand; P folds
into the tiny stationary factor A^T @ stat per call.

Work distribution: the unit is a chain (i, sg2, ccol) — the four slabs
j = 4*(sg2+4m)+ccol that one on-device reduction lane sums.  For each
of the 16 (sg2, ccol) positions the 64 i-chains are dealt across the
8 cores by descending weight (rank r -> core r%8, row-group r//8), so
the near-full-rank i=0 slabs spread evenly and the SPMD contract
depths (max over cores) stay tight.  The host adds the final 16
partial groups per output row using the chain map.

Device per core and row-group: 64 col-tiled K x 32 x 128 matmuls (4
PE column groups concurrent) fill PSUM [4*32 l-bands, 16 slots x 128
h]; the DVE multiplies PSUM halves by the token embeddings (fp16);
gpsimd + DVE fold 16 slots into 4 groups; [128, 512] fp16 rows DMA
out.
"""
import numpy as np
import ml_dtypes

import concourse.bacc as bacc
import concourse.tile as tile
from concourse import mybir
from concourse.bass_utils import run_bass_kernel_spmd

F32 = mybir.dt.float32
BF16 = mybir.dt.bfloat16
FP16 = mybir.dt.float16
TWO_PI = 2.0 * np.pi

M, O, B = 64, 16, 4096
NCORES = 8
NI = 8            # row-groups per core
NH, NL = 128, 32  # b = 32*h + l
TAU = 0.01        # relative Frobenius tail kept when truncating mov

_bf16 = ml_dtypes.bfloat16
_fp16 = np.float16
_nc_cache = {}
_last_results = None


def _factors():
    """P-independent SVD factors, the chain map, and contract depths.

    imap[c, rg, sg2, ccol] = i_global handled by core c, row-group rg,
    reduction lane (sg2, ccol).  KPROG[rg, j] = contract depth of the
    matmul at (rg, slot=j//4, ccol=j%4) — max slab rank over cores.
    """
    if "fac" in _nc_cache:
        return _nc_cache["fac"]
    h = np.arange(NH, dtype=np.float64)
    ig = np.arange(M, dtype=np.float64)[:, None, None]
    jg = np.arange(M, dtype=np.float64)[None, :, None]
    gg = np.arange(O, dtype=np.float64)[None, None, :]
    theta = TWO_PI / (1024.0 * ig + 16.0 * jg + gg + 2.0)
    a1 = theta[..., None] * (32.0 * h)
    mov = np.concatenate([np.cos(a1), np.sin(a1)], axis=2).reshape(M * M, 32, NH)
    U, S, Vt = np.linalg.svd(mov.astype(np.float64), full_matrices=False)
    fro = np.sqrt((S ** 2).sum(1))
    tail = np.sqrt(np.cumsum((S ** 2)[:, ::-1], axis=1))[:, ::-1] / fro[:, None]
    Ks = np.maximum(
        np.array([np.searchsorted(-tail[s], -TAU) for s in range(M * M)]), 1)
    A = (U * S[:, None, :]).astype(np.float32)              # [4096, 32, 32]

    Ksq = Ks.reshape(M, M)                                  # [i, j]
    imap = np.zeros((NCORES, NI, 4, 4), dtype=np.int64)
    for sg2 in range(4):
        for ccol in range(4):
            js = [4 * (sg2 + 4 * m) + ccol for m in range(4)]
            w = Ksq[:, js].sum(axis=1)                      # weight per i
            order = np.argsort(-w, kind="stable")           # heavy first
            for r, i in enumerate(order):
                imap[r % NCORES, r // NCORES, sg2, ccol] = i
    KPROG = np.zeros((NI, M), dtype=np.int64)               # [rg, j]
    for rg in range(NI):
        for j in range(M):
            sg2, ccol = (j // 4) % 4, j % 4
            KPROG[rg, j] = max(Ksq[imap[c, rg, sg2, ccol], j]
                               for c in range(NCORES))
    fac = (A, Vt.astype(np.float32), Ksq, imap, KPROG)
    _nc_cache["fac"] = fac
    return fac


def _build():
    if "nc" in _nc_cache:
        return _nc_cache["nc"]
    _, _, _, _, KPROG = _factors()
    nc = bacc.Bacc(target_bir_lowering=False, debug=False)
    wt_d = nc.declare_dram_parameter("wt", [32, 65536], BF16, isOutput=False)
    vt_d = nc.declare_dram_parameter("vt", [32, 16384], BF16, isOutput=False)
    xa_d = nc.declare_dram_parameter("xa", [128, 2048], FP16, isOutput=False)
    out_d = nc.declare_dram_parameter("out", [1024, 512], FP16, isOutput=True)

    with tile.TileContext(nc) as tc:
        with (
            tc.tile_pool(name="xap", bufs=1) as xpool,
            tc.tile_pool(name="wv", bufs=3) as wpool,
            tc.tile_pool(name="tmp", bufs=3) as tpool,
            tc.tile_pool(name="red", bufs=3) as rpool,
            tc.tile_pool(name="ps", bufs=4, space="PSUM") as psum,
        ):
            rgorder = [1, 2, 3, 4, 5, 6, 7, 0]
            xa = xpool.tile([128, 2048], FP16)
            wt_t, vt_t = {}, {}
            for n, rg in enumerate(rgorder):
                v = wpool.tile([32, 2048], BF16, name=f"vt{rg}", tag="vt")
                w = wpool.tile([32, 8192], BF16, name=f"wt{rg}", tag="wt")
                Kq = KPROG[rg].reshape(4, 16).max(axis=1)   # per quarter
                for q in range(4):
                    K = int(Kq[q])
                    nc.sync.dma_start(
                        v[0:K, 512 * q:512 * (q + 1)],
                        vt_d[0:K, 2048 * rg + 512 * q:
                             2048 * rg + 512 * (q + 1)])
                    nc.sync.dma_start(
                        w[0:K, 2048 * q:2048 * (q + 1)],
                        wt_d[0:K, 8192 * rg + 2048 * q:
                             8192 * rg + 2048 * (q + 1)])
                if n == 0:
                    nc.sync.dma_start(xa[:], xa_d[:])
                vt_t[rg] = v
                wt_t[rg] = w

            for n, rg in enumerate(rgorder):
                # 64 col-tiled matmuls per row-group (4 PE column groups
                # run concurrently; row-band tiling is rejected by this
                # HW path).  j = 4*slot + ccol.
                tx = tpool.tile([128, 2048], FP16, name=f"tx{rg}", tag="tx")
                for half in range(2):
                    ps = psum.tile([128, 1024], F32, tag="ps",
                                   name=f"ps{rg}_{half}")
                    for sh in range(8):
                        slot = 8 * half + sh
                        for ccol in range(4):
                            j = 4 * slot + ccol
                            K = int(KPROG[rg][j])
                            nc.tensor.matmul(
                                ps[32 * ccol:32 * ccol + 32,
                                   128 * sh:128 * sh + 128],
                                vt_t[rg][0:K, 32 * j:32 * j + 32],
                                wt_t[rg][0:K, 128 * j:128 * j + 128],
                                start=True, stop=True,
                                tile_position=(0, 32 * ccol))
                    nc.vector.tensor_tensor(
                        tx[:, 1024 * half:1024 * (half + 1)], ps[:, :],
                        xa[:, 1024 * half:1024 * (half + 1)],
                        mybir.AluOpType.mult)
                t1 = rpool.tile([128, 1024], FP16, name=f"t1_{rg}", tag="t1")
                nc.gpsimd.tensor_tensor(t1[:], tx[:, 0:1024],
                                        tx[:, 1024:2048],
                                        mybir.AluOpType.add)
                t2 = rpool.tile([128, 512], FP16, name=f"t2_{rg}", tag="t2")
                nc.vector.tensor_tensor(t2[:], t1[:, 0:512], t1[:, 512:1024],
                                        mybir.AluOpType.add)
                nc.sync.dma_start(out_d[128 * rg:128 * (rg + 1), :], t2[:])
    nc.compile()
    _nc_cache["nc"] = nc
    return nc


def _pack_tables(P_):
    """Per-core bf16 tables following the chain map.  Slab at (rg, j)
    of core c is (i = imap[c, rg, (j//4)%4, j%4], j); rows K..KPROG are
    zero."""
    A, Vt, Ksq, imap, KPROG = _factors()
    l = np.arange(NL, dtype=np.float64)
    ig = np.arange(M, dtype=np.float64)[:, None, None]
    jg = np.arange(M, dtype=np.float64)[None, :, None]
    gg = np.arange(O, dtype=np.float64)[None, None, :]
    theta = TWO_PI / (1024.0 * ig + 16.0 * jg + gg + 2.0)
    a2 = theta[..., None] * l
    Pd = P_.astype(np.float64)
    stat = np.concatenate([Pd[..., None] * np.cos(a2),
                           -Pd[..., None] * np.sin(a2)],
                          axis=2).reshape(M * M, 32, NL).astype(np.float32)
    statp = np.matmul(A.transpose(0, 2, 1), stat)            # [4096,32,32]
    wts, vts = [], []
    for c in range(NCORES):
        wt = np.zeros((32, NI * 8192), dtype=_bf16)
        vt = np.zeros((32, NI * 2048), dtype=_bf16)
        for rg in range(NI):
            for j in range(M):
                i = imap[c, rg, (j // 4) % 4, j % 4]
                s = i * M + j
                K = int(Ksq[i, j])
                wt[0:K, 8192 * rg + NH * j:8192 * rg + NH * (j + 1)] = \
                    Vt[s][0:K].astype(_bf16)
                vt[0:K, 2048 * rg + NL * j:2048 * rg + NL * (j + 1)] = \
                    statp[s][0:K].astype(_bf16)
        wts.append(wt)
        vts.append(vt)
    return wts, vts


def _pack_x(x):
    # xa[32*ccol + l, 128*s + h] = x[32h+l, j], j = 4*s + ccol
    x4 = x.reshape(NH, NL, 16, 4)                 # [h, l, s, ccol]
    xa = np.ascontiguousarray(x4.transpose(3, 1, 2, 0)).reshape(128, 2048)
    return xa.astype(_fp16)


def _numpy_fallback(k, x, P_):
    out = np.zeros((B, M), dtype=np.float32)
    periods = (np.arange(M * M * O, dtype=np.float32) + 2.0).reshape(M, M, O)
    CH = 256
    for s0 in range(0, B, CH):
        kb = k[s0:s0 + CH].astype(np.float32)
        phi = np.cos(np.float32(TWO_PI) * kb[:, None, None, None]
                     / periods[None]).astype(np.float32)
        out[s0:s0 + CH] = np.einsum('bj,ijg,bijg->bi', x[s0:s0 + CH],
                                    P_.astype(np.float32), phi,
                                    optimize=True).astype(np.float32)
    return out


def kernel(k_tensor, token_indices, emb, P):
    global _last_results
    k = np.asarray(k_tensor, dtype=np.float32).reshape(B)
    tok = np.asarray(token_indices).astype(np.int64).reshape(B)
    emb_ = np.asarray(emb, dtype=np.float32)
    P_ = np.asarray(P, dtype=np.float32)
    x = emb_[tok]                                          # [B, 64]

    if not np.array_equal(k, np.arange(B, dtype=np.float32)):
        return _numpy_fallback(k, x, P_)

    _, _, _, imap, _ = _factors()
    wts, vts = _pack_tables(P_)
    xa = _pack_x(x)
    nc = _build()
    in_maps = [{"wt": wts[c], "vt": vts[c], "xa": xa} for c in range(NCORES)]
    res = run_bass_kernel_spmd(nc, in_maps, list(range(NCORES)))
    _last_results = res
    out = np.zeros((B, M), dtype=np.float32)
    for c in range(NCORES):
        od = res.results[c]["out"].astype(np.float32)       # [1024, 512]
        # row = 128*rg + 32*ccol + l; col = 128*sg2 + h
        part = od.reshape(NI, 4, NL, 4, NH)                 # [rg,ccol,l,sg2,h]
        for rg in range(NI):
            for ccol in range(4):
                for sg2 in range(4):
                    i = imap[c, rg, sg2, ccol]
                    out[:, i] += part[rg, ccol, :, sg2, :].T.reshape(B)
    return out


# revision 19
# speedup vs baseline: 1.0212x; 1.0212x over previous
"""Trainium2 Bass kernel for nn_DualDescriptorTS.

Math:  Nk[b,i] = sum_{j,g} x[b,j] * P[i,j,g] * cos(2*pi*k[b]/p[i,j,g]),
       p[i,j,g] = i*1024 + j*16 + g + 2,  x = emb[token_indices].

Key identity (k = arange(B), so k_b = b = 32*h + l, h in [0,128),
l in [0,32)): by angle addition, the P-weighted phi slab of each
(i, j) pair is a small-rank product

  D_{i,j}[l, h] = stat^T mov,
  stat[(c,g), l] = {P*cos(l*th_g), -P*sin(l*th_g)},  mov[(c,g), h] =
  {cos(32h*th_g), sin(32h*th_g)},  th_g = 2*pi/p.

mov is P-independent and numerically low rank (the 16 periods of a
slab are nearly equal): mov ~ A @ Q with per-slab rank K (median 2 at
a 1% Frobenius tail).  Q ships as the matmul moving operand; P folds
into the tiny stationary factor A^T @ stat per call.

Work distribution: the unit is a chain (i, sg2, ccol) — the four slabs
j = 4*(sg2+4m)+ccol that one on-device reduction lane sums.  For each
of the 16 (sg2, ccol) positions the 64 i-chains are dealt across the
8 cores by descending weight (rank r -> core r%8, row-group r//8), so
the near-full-rank i=0 slabs spread evenly and the SPMD contract
depths (max over cores) stay tight.  The host adds the final 16
partial groups per output row using the chain map.

Device per core and row-group: 64 col-tiled K x 32 x 128 matmuls (4
PE column groups concurrent) fill PSUM [4*32 l-bands, 16 slots x 128
h]; the DVE multiplies PSUM halves by the token embeddings (fp16);
gpsimd + DVE fold 16 slots into 4 groups; [128, 512] fp16 rows DMA
out.
"""
import numpy as np
import ml_dtypes

import concourse.bacc as bacc
import concourse.tile as tile
from concourse import mybir
from concourse.bass_utils import run_bass_kernel_spmd

F32 = mybir.dt.float32
BF16 = mybir.dt.bfloat16
FP16 = mybir.dt.float16
TWO_PI = 2.0 * np.pi

M, O, B = 64, 16, 4096
NCORES = 8
NI = 8            # row-groups per core
NH, NL = 128, 32  # b = 32*h + l
TAU = 0.01        # relative Frobenius tail kept when truncating mov

_bf16 = ml_dtypes.bfloat16
_fp16 = np.float16
_nc_cache = {}
_last_results = None


def _factors():
    """P-independent SVD factors, the chain map, and contract depths.

    imap[c, rg, sg2, ccol] = i_global handled by core c, row-group rg,
    reduction lane (sg2, ccol).  KPROG[rg, j] = contract depth of the
    matmul at (rg, slot=j//4, ccol=j%4) — max slab rank over cores.
    """
    if "fac" in _nc_cache:
        return _nc_cache["fac"]
    h = np.arange(NH, dtype=np.float64)
    ig = np.arange(M, dtype=np.float64)[:, None, None]
    jg = np.arange(M, dtype=np.float64)[None, :, None]
    gg = np.arange(O, dtype=np.float64)[None, None, :]
    theta = TWO_PI / (1024.0 * ig + 16.0 * jg + gg + 2.0)
    a1 = theta[..., None] * (32.0 * h)
    mov = np.concatenate([np.cos(a1), np.sin(a1)], axis=2).reshape(M * M, 32, NH)
    U, S, Vt = np.linalg.svd(mov.astype(np.float64), full_matrices=False)
    fro = np.sqrt((S ** 2).sum(1))
    tail = np.sqrt(np.cumsum((S ** 2)[:, ::-1], axis=1))[:, ::-1] / fro[:, None]
    Ks = np.maximum(
        np.array([np.searchsorted(-tail[s], -TAU) for s in range(M * M)]), 1)
    A = (U * S[:, None, :]).astype(np.float32)              # [4096, 32, 32]

    Ksq = Ks.reshape(M, M)                                  # [i, j]
    imap = np.zeros((NCORES, NI, 4, 4), dtype=np.int64)
    for sg2 in range(4):
        for ccol in range(4):
            js = [4 * (sg2 + 4 * m) + ccol for m in range(4)]
            w = Ksq[:, js].sum(axis=1)                      # weight per i
            order = np.argsort(-w, kind="stable")           # heavy first
            for r, i in enumerate(order):
                imap[r % NCORES, r // NCORES, sg2, ccol] = i
    KPROG = np.zeros((NI, M), dtype=np.int64)               # [rg, j]
    for rg in range(NI):
        for j in range(M):
            sg2, ccol = (j // 4) % 4, j % 4
            KPROG[rg, j] = max(Ksq[imap[c, rg, sg2, ccol], j]
                               for c in range(NCORES))
    fac = (A, Vt.astype(np.float32), Ksq, imap, KPROG)
    _nc_cache["fac"] = fac
    return fac


def _build():
    if "nc" in _nc_cache:
        return _nc_cache["nc"]
    _, _, _, _, KPROG = _factors()
    nc = bacc.Bacc(target_bir_lowering=False, debug=False)
    wt_d = nc.declare_dram_parameter("wt", [32, 65536], BF16, isOutput=False)
    vt_d = nc.declare_dram_parameter("vt", [32, 16384], BF16, isOutput=False)
    xa_d = nc.declare_dram_parameter("xa", [128, 2048], FP16, isOutput=False)
    out_d = nc.declare_dram_parameter("out", [1024, 512], FP16, isOutput=True)

    with tile.TileContext(nc) as tc:
        with (
            tc.tile_pool(name="xap", bufs=1) as xpool,
            tc.tile_pool(name="wv", bufs=1) as wpool,
            tc.tile_pool(name="tmp", bufs=3) as tpool,
            tc.tile_pool(name="red", bufs=3) as rpool,
            tc.tile_pool(name="ps", bufs=4, space="PSUM") as psum,
        ):
            rgorder = [1, 2, 3, 4, 5, 6, 7, 0]
            xa = xpool.tile([128, 2048], FP16)
            # Five input transfers on three issue queues: DMA issue costs
            # ~700ns serially per engine, so light rgs 1..7 ship as one
            # strided block at their max contract depth.
            Rl = int(KPROG[1:].max())
            Rh = int(KPROG[0].max())
            wl = wpool.tile([32, 7 * 8192], BF16, name="wl")
            vl = wpool.tile([32, 7 * 2048], BF16, name="vl")
            wh = wpool.tile([32, 8192], BF16, name="wh")
            vh = wpool.tile([32, 2048], BF16, name="vh")
            nc.sync.dma_start(wl[0:Rl, :], wt_d[0:Rl, 8192:65536])
            nc.scalar.dma_start(vl[0:Rl, :], vt_d[0:Rl, 2048:16384])
            nc.gpsimd.dma_start(xa[:], xa_d[:])
            nc.sync.dma_start(wh[0:Rh, :], wt_d[0:Rh, 0:8192])
            nc.scalar.dma_start(vh[0:Rh, :], vt_d[0:Rh, 0:2048])
            wt_t = {0: (wh, 0)}
            vt_t = {0: (vh, 0)}
            for rg in range(1, NI):
                wt_t[rg] = (wl, 8192 * (rg - 1))
                vt_t[rg] = (vl, 2048 * (rg - 1))

            for n, rg in enumerate(rgorder):
                # 64 col-tiled matmuls per row-group (4 PE column groups
                # run concurrently; row-band tiling is rejected by this
                # HW path).  j = 4*slot + ccol.
                tx = tpool.tile([128, 2048], FP16, name=f"tx{rg}", tag="tx")
                for half in range(2):
                    ps = psum.tile([128, 1024], F32, tag="ps",
                                   name=f"ps{rg}_{half}")
                    for sh in range(8):
                        slot = 8 * half + sh
                        for ccol in range(4):
                            j = 4 * slot + ccol
                            K = int(KPROG[rg][j])
                            vtile, voff = vt_t[rg]
                            wtile, woff = wt_t[rg]
                            nc.tensor.matmul(
                                ps[32 * ccol:32 * ccol + 32,
                                   128 * sh:128 * sh + 128],
                                vtile[0:K, voff + 32 * j:voff + 32 * j + 32],
                                wtile[0:K, woff + 128 * j:
                                      woff + 128 * j + 128],
                                start=True, stop=True,
                                tile_position=(0, 32 * ccol))
                    nc.vector.tensor_tensor(
                        tx[:, 1024 * half:1024 * (half + 1)], ps[:, :],
                        xa[:, 1024 * half:1024 * (half + 1)],
                        mybir.AluOpType.mult)
                t1 = rpool.tile([128, 1024], FP16, name=f"t1_{rg}", tag="t1")
                nc.gpsimd.tensor_tensor(t1[:], tx[:, 0:1024],
                                        tx[:, 1024:2048],
                                        mybir.AluOpType.add)
                t2 = rpool.tile([128, 512], FP16, name=f"t2_{rg}", tag="t2")
                nc.vector.tensor_tensor(t2[:], t1[:, 0:512], t1[:, 512:1024],
                                        mybir.AluOpType.add)
                nc.scalar.dma_start(out_d[128 * rg:128 * (rg + 1), :], t2[:])
    nc.compile()
    _nc_cache["nc"] = nc
    return nc


def _pack_tables(P_):
    """Per-core bf16 tables following the chain map.  Slab at (rg, j)
    of core c is (i = imap[c, rg, (j//4)%4, j%4], j); rows K..KPROG are
    zero."""
    A, Vt, Ksq, imap, KPROG = _factors()
    l = np.arange(NL, dtype=np.float64)
    ig = np.arange(M, dtype=np.float64)[:, None, None]
    jg = np.arange(M, dtype=np.float64)[None, :, None]
    gg = np.arange(O, dtype=np.float64)[None, None, :]
    theta = TWO_PI / (1024.0 * ig + 16.0 * jg + gg + 2.0)
    a2 = theta[..., None] * l
    Pd = P_.astype(np.float64)
    stat = np.concatenate([Pd[..., None] * np.cos(a2),
                           -Pd[..., None] * np.sin(a2)],
                          axis=2).reshape(M * M, 32, NL).astype(np.float32)
    statp = np.matmul(A.transpose(0, 2, 1), stat)            # [4096,32,32]
    wts, vts = [], []
    for c in range(NCORES):
        wt = np.zeros((32, NI * 8192), dtype=_bf16)
        vt = np.zeros((32, NI * 2048), dtype=_bf16)
        for rg in range(NI):
            for j in range(M):
                i = imap[c, rg, (j // 4) % 4, j % 4]
                s = i * M + j
                K = int(Ksq[i, j])
                wt[0:K, 8192 * rg + NH * j:8192 * rg + NH * (j + 1)] = \
                    Vt[s][0:K].astype(_bf16)
                vt[0:K, 2048 * rg + NL * j:2048 * rg + NL * (j + 1)] = \
                    statp[s][0:K].astype(_bf16)
        wts.append(wt)
        vts.append(vt)
    return wts, vts


def _pack_x(x):
    # xa[32*ccol + l, 128*s + h] = x[32h+l, j], j = 4*s + ccol
    x4 = x.reshape(NH, NL, 16, 4)                 # [h, l, s, ccol]
    xa = np.ascontiguousarray(x4.transpose(3, 1, 2, 0)).reshape(128, 2048)
    return xa.astype(_fp16)


def _numpy_fallback(k, x, P_):
    out = np.zeros((B, M), dtype=np.float32)
    periods = (np.arange(M * M * O, dtype=np.float32) + 2.0).reshape(M, M, O)
    CH = 256
    for s0 in range(0, B, CH):
        kb = k[s0:s0 + CH].astype(np.float32)
        phi = np.cos(np.float32(TWO_PI) * kb[:, None, None, None]
                     / periods[None]).astype(np.float32)
        out[s0:s0 + CH] = np.einsum('bj,ijg,bijg->bi', x[s0:s0 + CH],
                                    P_.astype(np.float32), phi,
                                    optimize=True).astype(np.float32)
    return out


def kernel(k_tensor, token_indices, emb, P):
    global _last_results
    k = np.asarray(k_tensor, dtype=np.float32).reshape(B)
    tok = np.asarray(token_indices).astype(np.int64).reshape(B)
    emb_ = np.asarray(emb, dtype=np.float32)
    P_ = np.asarray(P, dtype=np.float32)
    x = emb_[tok]                                          # [B, 64]

    if not np.array_equal(k, np.arange(B, dtype=np.float32)):
        return _numpy_fallback(k, x, P_)

    _, _, _, imap, _ = _factors()
    wts, vts = _pack_tables(P_)
    xa = _pack_x(x)
    nc = _build()
    in_maps = [{"wt": wts[c], "vt": vts[c], "xa": xa} for c in range(NCORES)]
    res = run_bass_kernel_spmd(nc, in_maps, list(range(NCORES)))
    _last_results = res
    out = np.zeros((B, M), dtype=np.float32)
    for c in range(NCORES):
        od = res.results[c]["out"].astype(np.float32)       # [1024, 512]
        # row = 128*rg + 32*ccol + l; col = 128*sg2 + h
        part = od.reshape(NI, 4, NL, 4, NH)                 # [rg,ccol,l,sg2,h]
        for rg in range(NI):
            for ccol in range(4):
                for sg2 in range(4):
                    i = imap[c, rg, sg2, ccol]
                    out[:, i] += part[rg, ccol, :, sg2, :].T.reshape(B)
    return out


# revision 21
# speedup vs baseline: 1.1463x; 1.1225x over previous
"""Trainium2 Bass kernel for nn_DualDescriptorTS.

Math:  Nk[b,i] = sum_{j,g} x[b,j] * P[i,j,g] * cos(2*pi*k[b]/p[i,j,g]),
       p[i,j,g] = i*1024 + j*16 + g + 2,  x = emb[token_indices].

Key identity (k = arange(B), so k_b = b = 32*h + l, h in [0,128),
l in [0,32)): by angle addition, the P-weighted phi slab of each
(i, j) pair is a small-rank product

  D_{i,j}[l, h] = stat^T mov,
  stat[(c,g), l] = {P*cos(l*th_g), -P*sin(l*th_g)},  mov[(c,g), h] =
  {cos(32h*th_g), sin(32h*th_g)},  th_g = 2*pi/p.

mov is P-independent and numerically low rank (the 16 periods of a
slab are nearly equal): mov ~ A @ Q with per-slab rank K (median 2 at
a 1% Frobenius tail).  Q ships as the matmul moving operand; P folds
into the tiny stationary factor A^T @ stat per call.

Work distribution: the unit is a chain (i, sg2, ccol) — the four slabs
j = 4*(sg2+4m)+ccol that one on-device reduction lane sums.  For each
of the 16 (sg2, ccol) positions the 64 i-chains are dealt across the
8 cores by descending weight (rank r -> core r%8, row-group r//8), so
the near-full-rank i=0 slabs spread evenly and the SPMD contract
depths (max over cores) stay tight.  The host adds the final 16
partial groups per output row using the chain map.

Device per core and row-group: 64 col-tiled K x 32 x 128 matmuls (4
PE column groups concurrent) fill PSUM [4*32 l-bands, 16 slots x 128
h]; the DVE multiplies PSUM halves by the token embeddings (fp16);
gpsimd + DVE fold 16 slots into 4 groups; [128, 512] fp16 rows DMA
out.
"""
import numpy as np
import ml_dtypes

import concourse.bacc as bacc
import concourse.tile as tile
from concourse import mybir
from concourse.bass_utils import run_bass_kernel_spmd

F32 = mybir.dt.float32
BF16 = mybir.dt.bfloat16
FP16 = mybir.dt.float16
TWO_PI = 2.0 * np.pi

M, O, B = 64, 16, 4096
NCORES = 8
NI = 8            # row-groups per core
NH, NL = 128, 32  # b = 32*h + l
TAU = 0.01        # relative Frobenius tail kept when truncating mov

_bf16 = ml_dtypes.bfloat16
_fp16 = np.float16
_nc_cache = {}
_last_results = None


def _factors():
    """P-independent SVD factors, the chain map, and contract depths.

    imap[c, rg, sg2, ccol] = i_global handled by core c, row-group rg,
    reduction lane (sg2, ccol).  KPROG[rg, j] = contract depth of the
    matmul at (rg, slot=j//4, ccol=j%4) — max slab rank over cores.
    """
    if "fac" in _nc_cache:
        return _nc_cache["fac"]
    h = np.arange(NH, dtype=np.float64)
    ig = np.arange(M, dtype=np.float64)[:, None, None]
    jg = np.arange(M, dtype=np.float64)[None, :, None]
    gg = np.arange(O, dtype=np.float64)[None, None, :]
    theta = TWO_PI / (1024.0 * ig + 16.0 * jg + gg + 2.0)
    a1 = theta[..., None] * (32.0 * h)
    mov = np.concatenate([np.cos(a1), np.sin(a1)], axis=2).reshape(M * M, 32, NH)
    U, S, Vt = np.linalg.svd(mov.astype(np.float64), full_matrices=False)
    fro = np.sqrt((S ** 2).sum(1))
    tail = np.sqrt(np.cumsum((S ** 2)[:, ::-1], axis=1))[:, ::-1] / fro[:, None]
    Ks = np.maximum(
        np.array([np.searchsorted(-tail[s], -TAU) for s in range(M * M)]), 1)
    A = (U * S[:, None, :]).astype(np.float32)              # [4096, 32, 32]

    Ksq = Ks.reshape(M, M)                                  # [i, j]
    imap = np.zeros((NCORES, NI, 4, 4), dtype=np.int64)
    for sg2 in range(4):
        for ccol in range(4):
            js = [4 * (sg2 + 4 * m) + ccol for m in range(4)]
            w = Ksq[:, js].sum(axis=1)                      # weight per i
            order = np.argsort(-w, kind="stable")           # heavy first
            for r, i in enumerate(order):
                imap[r % NCORES, r // NCORES, sg2, ccol] = i
    KPROG = np.zeros((NI, M), dtype=np.int64)               # [rg, j]
    for rg in range(NI):
        for j in range(M):
            sg2, ccol = (j // 4) % 4, j % 4
            KPROG[rg, j] = max(Ksq[imap[c, rg, sg2, ccol], j]
                               for c in range(NCORES))
    fac = (A, Vt.astype(np.float32), Ksq, imap, KPROG)
    _nc_cache["fac"] = fac
    return fac


def _build():
    if "nc" in _nc_cache:
        return _nc_cache["nc"]
    _, _, _, _, KPROG = _factors()
    nc = bacc.Bacc(target_bir_lowering=False, debug=False)
    wt_d = nc.declare_dram_parameter("wt", [32, 65536], BF16, isOutput=False)
    vt_d = nc.declare_dram_parameter("vt", [32, 16384], BF16, isOutput=False)
    xa_d = nc.declare_dram_parameter("xa", [128, 2048], FP16, isOutput=False)
    out_d = nc.declare_dram_parameter("out", [1024, 512], FP16, isOutput=True)

    with tile.TileContext(nc) as tc:
        with (
            tc.tile_pool(name="xap", bufs=1) as xpool,
            tc.tile_pool(name="wv", bufs=1) as wpool,
            tc.tile_pool(name="tmp", bufs=3) as tpool,
            tc.tile_pool(name="red", bufs=3) as rpool,
            tc.tile_pool(name="ps", bufs=4, space="PSUM") as psum,
        ):
            rgorder = [1, 2, 3, 4, 5, 6, 7, 0]
            xa = xpool.tile([128, 2048], FP16)
            # Five input transfers on three issue queues: DMA issue costs
            # ~700ns serially per engine, so light rgs 1..7 ship as one
            # strided block at their max contract depth.
            Rh = int(KPROG[0].max())
            wl = wpool.tile([32, 7 * 8192], BF16, name="wl")
            vl = wpool.tile([32, 7 * 2048], BF16, name="vl")
            wh = wpool.tile([32, 8192], BF16, name="wh")
            vh = wpool.tile([32, 2048], BF16, name="vh")
            # staged light chunks so rg1 starts early; one issue each on
            # two queues (issue ~700ns is serial per engine)
            for rgs in ([1], [2, 3], [4, 5, 6, 7]):
                Rc = int(KPROG[rgs].max())
                a, b = rgs[0] - 1, rgs[-1]
                nc.sync.dma_start(wl[0:Rc, 8192 * a:8192 * b],
                                  wt_d[0:Rc, 8192 * (a + 1):8192 * (b + 1)])
                nc.scalar.dma_start(vl[0:Rc, 2048 * a:2048 * b],
                                    vt_d[0:Rc, 2048 * (a + 1):2048 * (b + 1)])
                if rgs[0] == 1:
                    nc.gpsimd.dma_start(xa[:], xa_d[:])
            nc.sync.dma_start(wh[0:Rh, :], wt_d[0:Rh, 0:8192])
            nc.scalar.dma_start(vh[0:Rh, :], vt_d[0:Rh, 0:2048])
            wt_t = {0: (wh, 0)}
            vt_t = {0: (vh, 0)}
            for rg in range(1, NI):
                wt_t[rg] = (wl, 8192 * (rg - 1))
                vt_t[rg] = (vl, 2048 * (rg - 1))

            for n, rg in enumerate(rgorder):
                # 64 col-tiled matmuls per row-group (4 PE column groups
                # run concurrently; row-band tiling is rejected by this
                # HW path).  j = 4*slot + ccol.
                tx = tpool.tile([128, 2048], FP16, name=f"tx{rg}", tag="tx")
                for half in range(2):
                    ps = psum.tile([128, 1024], F32, tag="ps",
                                   name=f"ps{rg}_{half}")
                    for sh in range(8):
                        slot = 8 * half + sh
                        for ccol in range(4):
                            j = 4 * slot + ccol
                            K = int(KPROG[rg][j])
                            vtile, voff = vt_t[rg]
                            wtile, woff = wt_t[rg]
                            nc.tensor.matmul(
                                ps[32 * ccol:32 * ccol + 32,
                                   128 * sh:128 * sh + 128],
                                vtile[0:K, voff + 32 * j:voff + 32 * j + 32],
                                wtile[0:K, woff + 128 * j:
                                      woff + 128 * j + 128],
                                start=True, stop=True,
                                tile_position=(0, 32 * ccol))
                    nc.vector.tensor_tensor(
                        tx[:, 1024 * half:1024 * (half + 1)], ps[:, :],
                        xa[:, 1024 * half:1024 * (half + 1)],
                        mybir.AluOpType.mult)
                t1 = rpool.tile([128, 1024], FP16, name=f"t1_{rg}", tag="t1")
                nc.gpsimd.tensor_tensor(t1[:], tx[:, 0:1024],
                                        tx[:, 1024:2048],
                                        mybir.AluOpType.add)
                t2 = rpool.tile([128, 512], FP16, name=f"t2_{rg}", tag="t2")
                eng2 = nc.vector if n % 2 == 0 else nc.gpsimd
                eng2.tensor_tensor(t2[:], t1[:, 0:512], t1[:, 512:1024],
                                   mybir.AluOpType.add)
                nc.scalar.dma_start(out_d[128 * rg:128 * (rg + 1), :], t2[:])
    nc.compile()
    _nc_cache["nc"] = nc
    return nc


def _pack_tables(P_):
    """Per-core bf16 tables following the chain map.  Slab at (rg, j)
    of core c is (i = imap[c, rg, (j//4)%4, j%4], j); rows K..KPROG are
    zero."""
    A, Vt, Ksq, imap, KPROG = _factors()
    l = np.arange(NL, dtype=np.float64)
    ig = np.arange(M, dtype=np.float64)[:, None, None]
    jg = np.arange(M, dtype=np.float64)[None, :, None]
    gg = np.arange(O, dtype=np.float64)[None, None, :]
    theta = TWO_PI / (1024.0 * ig + 16.0 * jg + gg + 2.0)
    a2 = theta[..., None] * l
    Pd = P_.astype(np.float64)
    stat = np.concatenate([Pd[..., None] * np.cos(a2),
                           -Pd[..., None] * np.sin(a2)],
                          axis=2).reshape(M * M, 32, NL).astype(np.float32)
    statp = np.matmul(A.transpose(0, 2, 1), stat)            # [4096,32,32]
    wts, vts = [], []
    for c in range(NCORES):
        wt = np.zeros((32, NI * 8192), dtype=_bf16)
        vt = np.zeros((32, NI * 2048), dtype=_bf16)
        for rg in range(NI):
            for j in range(M):
                i = imap[c, rg, (j // 4) % 4, j % 4]
                s = i * M + j
                K = int(Ksq[i, j])
                wt[0:K, 8192 * rg + NH * j:8192 * rg + NH * (j + 1)] = \
                    Vt[s][0:K].astype(_bf16)
                vt[0:K, 2048 * rg + NL * j:2048 * rg + NL * (j + 1)] = \
                    statp[s][0:K].astype(_bf16)
        wts.append(wt)
        vts.append(vt)
    return wts, vts


def _pack_x(x):
    # xa[32*ccol + l, 128*s + h] = x[32h+l, j], j = 4*s + ccol
    x4 = x.reshape(NH, NL, 16, 4)                 # [h, l, s, ccol]
    xa = np.ascontiguousarray(x4.transpose(3, 1, 2, 0)).reshape(128, 2048)
    return xa.astype(_fp16)


def _numpy_fallback(k, x, P_):
    out = np.zeros((B, M), dtype=np.float32)
    periods = (np.arange(M * M * O, dtype=np.float32) + 2.0).reshape(M, M, O)
    CH = 256
    for s0 in range(0, B, CH):
        kb = k[s0:s0 + CH].astype(np.float32)
        phi = np.cos(np.float32(TWO_PI) * kb[:, None, None, None]
                     / periods[None]).astype(np.float32)
        out[s0:s0 + CH] = np.einsum('bj,ijg,bijg->bi', x[s0:s0 + CH],
                                    P_.astype(np.float32), phi,
                                    optimize=True).astype(np.float32)
    return out


def kernel(k_tensor, token_indices, emb, P):
    global _last_results
    k = np.asarray(k_tensor, dtype=np.float32).reshape(B)
    tok = np.asarray(token_indices).astype(np.int64).reshape(B)
    emb_ = np.asarray(emb, dtype=np.float32)
    P_ = np.asarray(P, dtype=np.float32)
    x = emb_[tok]                                          # [B, 64]

    if not np.array_equal(k, np.arange(B, dtype=np.float32)):
        return _numpy_fallback(k, x, P_)

    _, _, _, imap, _ = _factors()
    wts, vts = _pack_tables(P_)
    xa = _pack_x(x)
    nc = _build()
    in_maps = [{"wt": wts[c], "vt": vts[c], "xa": xa} for c in range(NCORES)]
    res = run_bass_kernel_spmd(nc, in_maps, list(range(NCORES)))
    _last_results = res
    out = np.zeros((B, M), dtype=np.float32)
    for c in range(NCORES):
        od = res.results[c]["out"].astype(np.float32)       # [1024, 512]
        # row = 128*rg + 32*ccol + l; col = 128*sg2 + h
        part = od.reshape(NI, 4, NL, 4, NH)                 # [rg,ccol,l,sg2,h]
        for rg in range(NI):
            for ccol in range(4):
                for sg2 in range(4):
                    i = imap[c, rg, sg2, ccol]
                    out[:, i] += part[rg, ccol, :, sg2, :].T.reshape(B)
    return out
